# revision 8
# baseline (speedup 1.0000x reference)
"""GCN layer (CustomGraphConv) on 8 trn2 NeuronCores via Bass/Tile.

Math: out = D^{-1/2} (A + I) D^{-1/2} @ X @ W + bias
  - A: [N, N] 0/1 symmetric adjacency (f32 input), N = 8192
  - X: [N, 256] f32, W: [256, 256] f32, bias: [256] f32

Sharding: 1D node partition. Core c owns nodes R_c = [c*C, (c+1)*C), C = N/8.
Each core receives the column strip A_aug[:, R_c] (A with self-loops added on
the host, cast to fp8e4 — exact for 0/1 values), the full X^T and W in fp16
(replicated), and the bias broadcast to a [128, 256] f32 tile.

Device program (identical SPMD program on all 8 cores):
  1. Load A strip into SBUF as even-sized packed [128, pk*C] fp8 tiles.
  2. deg = colsum(strip) on PE in fp8 DoubleRow mode (ones [128,2,128] fp8
     stationary x A k-tile pairs moving): 0.5 cycles/row, 2 k-tiles per MM.
  3. AllGather degree shards -> deg_full [N]; d = 1/sqrt(deg). The whole
     degree exchange runs on the Pool queue so it is never blocked behind
     X-chunk DMAs on the SP queue.
  4. Z = X @ W via PE (X^T chunks stationary, W moving), fp16, drained to a
     single resident z_all strip.
  5. Per k-tile prep (streams ahead of the main matmul):
       z8[m] = fp8(64*d*Z[m])            (ACT, activation Copy with scale)
       rp[m] = fp8(64*d*Z[m] - z8[m])    (DVE, scalar_tensor_tensor)
     The x64 keeps the residual out of the fp8 subnormal floor; both carry
     the SAME scale so their matmuls share one PSUM accumulation group.
  6. Main matmul in DoubleRow fp8, k-pair outer / jt inner with all 8 PSUM
     banks open (one group per jt): psum[jt] += A_pair^T @ z8_pair
     + A_pair^T @ rp_pair. Streams behind the prep of step 5.
  7. Epilogue per jt: out = (d_own/64) * psum + bias via one DVE
     scalar_tensor_tensor; per-jt stores.

Toolchain constraints discovered on this stack:
  - walrus rejects >1 semaphore wait per instruction -> _split_dma_waits
    hoists extras onto standalone EventSemaphore instructions.
  - SBUF access patterns must keep the partition dim explicit: t[0, :]
    (rank-dropped) misdrives the DMA; use t[0:1, :].
  - A matmul accumulation group must own its PSUM bank exclusively until
    `stop` (start=True clears the whole 2KB bank); PSUM tiles are
    bank-rounded so one [128, 256] f32 tile = one bank.
  - fp8 is exact for 0/1 adjacency values; DoubleRow needs BOTH operands
    fp8e4/e5 with 3D [K, 2, free] APs and halves out partition/free vs the
    operand free sizes.
"""

import numpy as np
import ml_dtypes

import concourse.bass as bass
import concourse.mybir as mybir
import concourse.tile as tile
from concourse.bass_utils import run_bass_kernel_spmd

NCORES = 8
F = 256

f32 = mybir.dt.float32
fp16 = mybir.dt.float16
fp8 = mybir.dt.float8e4

RES_SCALE = 64.0  # unified x64 scale for z8 + residual (both in one psum group)


def _split_dma_waits(nc):
    """Hoist semaphore waits onto standalone EventSemaphore instructions on
    the issuing engine's queue, for any instruction carrying more than one.

    This toolchain's walrus caps sync waits at 1 per instruction (2 for
    EventSemaphore). A sequencer executes an attached wait and a preceding
    standalone wait identically, so hoisting preserves semantics (raw-bass
    wait_ge emits exactly this instruction).
    """
    ctr = 0
    for fn in nc.m.functions:
        for bb in fn.blocks:
            new_insts = []
            for inst in bb.instructions:
                si = inst.sync_info
                if (
                    not isinstance(inst, mybir.InstEventSemaphore)
                    and si is not None
                    and len(si.on_wait) > 1
                ):
                    for w in si.on_wait[:-1]:
                        ev = mybir.InstEventSemaphore(
                            name=f"hoistw-{ctr}",
                            engine=inst.engine,
                            ins=[],
                            outs=[],
                            sync_info=mybir.SyncInfo(on_wait=[w], on_update=[]),
                        )
                        ctr += 1
                        new_insts.append(ev)
                    inst.sync_info = mybir.SyncInfo(
                        on_wait=[si.on_wait[-1]], on_update=si.on_update
                    )
                new_insts.append(inst)
            bb.instructions = new_insts


def build(n_nodes: int, debug: bool = False, split_waits: bool = True):
    """Build the SPMD Bass program for one core (all cores identical)."""
    N = n_nodes
    C = N // NCORES  # own nodes per core
    KT = N // 128  # 128-row k tiles of the strip
    KP = KT // 2  # DoubleRow k-tile pairs
    JT = C // 128  # 128-col j tiles (own-node blocks)
    DEG_W = min(C, 512)
    DEG_CH = C // DEG_W  # colsum free-dim chunks (<=512 each)
    XCH = max(1, N // 1024)  # X^T column chunks
    XW = N // XCH  # columns per chunk
    MPC = XW // 128  # m tiles per X^T chunk

    nc = bass.Bass()
    a_strip = nc.dram_tensor("a_strip", [N, C], fp8, kind="ExternalInput")
    xt = nc.dram_tensor("xt", [F, N], fp16, kind="ExternalInput")
    w = nc.dram_tensor("w", [F, F], fp16, kind="ExternalInput")
    bias_bc = nc.dram_tensor("bias_bc", [128, F], f32, kind="ExternalInput")
    out = nc.dram_tensor("out", [C, F], f32, kind="ExternalOutput")
    if debug:
        deg_dump = nc.dram_tensor("deg_dump", [128, KT], f32, kind="ExternalOutput")
        z_dump = nc.dram_tensor("z_dump", [N, F], f32, kind="ExternalOutput")

    with tile.TileContext(nc) as tc:
        with (
            tc.tile_pool(name="persist", bufs=1) as persist,
            tc.tile_pool(name="work", bufs=2) as work,
            tc.tile_pool(name="dram", bufs=1, space="DRAM") as dram,
        ):
            # ---- write-once persistent loads ----
            # Even pack sizes keep DoubleRow k-tile pairs inside one pack.
            # Small leading packs start the colsum (which gates the
            # collective -> critical path) as early as possible.
            pack_sizes = [2, 2] + [4] * ((KT - 4) // 4)
            assert sum(pack_sizes) == KT
            a_pk = []
            pair2view = []  # kp -> (pack idx, pair offset within pack)
            k0 = 0
            for g, pk in enumerate(pack_sizes):
                t = persist.tile([128, pk * C], fp8, name=f"a{g}")
                a_pk.append(t)
                nc.sync.dma_start(
                    out=t.rearrange("p (t c) -> p t c", t=pk),
                    in_=a_strip[k0 * 128 : (k0 + pk) * 128, :].rearrange(
                        "(t p) c -> p t c", p=128
                    ),
                )
                for i in range(pk // 2):
                    pair2view.append((g, i))
                k0 += pk

            def a_pair(kp, lo, hi):
                """[128, 2, hi-lo] fp8 view of k-tile pair kp, cols [lo, hi)."""
                g, i = pair2view[kp]
                pk = pack_sizes[g]
                return a_pk[g].rearrange("p (t c) -> p t c", t=pk)[
                    :, 2 * i : 2 * i + 2, lo:hi
                ]

            w_sb = [persist.tile([128, F], fp16, name=f"w{i}") for i in range(2)]
            for i in range(2):
                nc.sync.dma_start(out=w_sb[i][:], in_=w[i * 128 : (i + 1) * 128, :])
            bias_sb = persist.tile([128, F], f32, name="bias")
            nc.sync.dma_start(out=bias_sb[:], in_=bias_bc[:])

            ones8 = persist.tile([128, 2 * 128], fp8, name="ones8")
            nc.vector.memset(ones8[:], 1.0)
            ones8v = ones8.rearrange("p (t c) -> p t c", t=2)

            # PSUM is 8 banks and pools reserve statically: the deg/z pools
            # are opened manually and closed before outpsum takes all 8.
            degpsum_cm = tc.tile_pool(name="degpsum", bufs=1, space="PSUM")
            degpsum = degpsum_cm.__enter__()
            zpsum_cm = tc.tile_pool(name="zpsum", bufs=2, space="PSUM")
            zpsum = zpsum_cm.__enter__()

            # ---- degrees of own nodes: DoubleRow colsum of the strip.
            # ones8 [128,2,128] stationary x A pair [128,2,DEG_W] moving;
            # every psum row = colsum over the 256 rows of the pair.
            deg_sb = persist.tile([1, C], f32, name="deg_sb")
            deg_ps = [
                degpsum.tile([128, DEG_W], f32, name=f"deg_ps{h}")
                for h in range(DEG_CH)
            ]
            last_cs_mm = None
            for kp in range(KP):
                for h in range(DEG_CH):
                    last_cs_mm = nc.tensor.matmul(
                        deg_ps[h][:],
                        ones8v,
                        a_pair(kp, h * DEG_W, (h + 1) * DEG_W),
                        start=(kp == 0),
                        stop=(kp == KP - 1),
                        perf_mode=mybir.MatmulPerfMode.DoubleRow,
                    )
            for h in range(DEG_CH):
                if h % 2 == 0:
                    nc.vector.tensor_copy(
                        deg_sb[:, h * DEG_W : (h + 1) * DEG_W], deg_ps[h][0:1, :]
                    )
                else:
                    nc.scalar.copy(
                        deg_sb[:, h * DEG_W : (h + 1) * DEG_W], deg_ps[h][0:1, :]
                    )

            # ---- gather degrees, d = 1/sqrt(deg), all on the Pool queue so
            # the exchange is never stuck behind X-chunk DMAs.
            cc_in = dram.tile([C], f32, name="cc_in")
            cc_out = dram.tile([N], f32, name="cc_out")
            nc.gpsimd.dma_start(
                out=cc_in.rearrange("(a b) -> a b", a=1), in_=deg_sb[0:1, :]
            )
            nc.gpsimd.collective_compute(
                "AllGather",
                mybir.AluOpType.bypass,
                replica_groups=[list(range(NCORES))],
                ins=[cc_in[:]],
                outs=[cc_out[:]],
            )
            deg_full = work.tile([128, KT], f32, tag="deg_full")
            nc.gpsimd.dma_start(
                out=deg_full[:], in_=cc_out.rearrange("(k p) -> p k", p=128)
            )
            d64 = persist.tile([128, KT], f32, name="d64")
            nc.vector.reciprocal(d64[:], deg_full[:])
            nc.scalar.sqrt(d64[:], d64[:])
            nc.vector.tensor_scalar_mul(d64[:], d64[:], float(RES_SCALE))

            # d_own/64 for the epilogue, from the local (pre-gather) degrees
            deg_own = work.tile([128, JT], f32, tag="deg_own")
            nc.gpsimd.dma_start(
                out=deg_own[:], in_=cc_in.rearrange("(j p) -> p j", p=128)
            )
            d_own = persist.tile([128, JT], f32, name="d_own")
            nc.vector.reciprocal(d_own[:], deg_own[:])
            nc.scalar.sqrt(d_own[:], d_own[:])
            nc.vector.tensor_scalar_mul(d_own[:], d_own[:], float(1.0 / RES_SCALE))

            if debug:
                nc.sync.dma_start(out=deg_dump[:], in_=deg_full[:])

            # ---- Z = X @ W (fp16 in, f32 accum, fp16 out), unscaled, into
            # one resident z_all strip. X^T streamed in write-once chunks.
            z_all = persist.tile([128, KT * F], fp16, name="z_all")
            for ch in range(XCH):
                xt_ch = [
                    work.tile(
                        [128, XW], fp16, name=f"xt_{ch}_{i}", tag=f"xt{i}", bufs=4
                    )
                    for i in range(2)
                ]
                for i in range(2):
                    nc.sync.dma_start(
                        out=xt_ch[i][:],
                        in_=xt[i * 128 : (i + 1) * 128, ch * XW : (ch + 1) * XW],
                    )
                for mi in range(MPC):
                    m = ch * MPC + mi
                    z_ps = zpsum.tile([128, F], f32, tag="z_ps")
                    for i in range(2):
                        mm = nc.tensor.matmul(
                            z_ps[:],
                            xt_ch[i][:, mi * 128 : (mi + 1) * 128],
                            w_sb[i][:],
                            start=(i == 0),
                            stop=(i == 1),
                        )
                        # keep Z-MMs off the PE until the colsum chain (which
                        # gates the collective -> critical path) is done
                        bass._add_dep_helper(
                            mm.ins, last_cs_mm.ins, reason="z after colsum"
                        )
                    # alternate psum->sbuf drain between DVE and ACT
                    zsl = z_all[:, m * F : (m + 1) * F]
                    if m % 2 == 0:
                        nc.vector.tensor_copy(zsl, z_ps[:])
                    else:
                        nc.scalar.copy(zsl, z_ps[:])

            zpsum_cm.__exit__(None, None, None)
            degpsum_cm.__exit__(None, None, None)
            outpsum_cm = tc.tile_pool(name="outpsum", bufs=1, space="PSUM")
            outpsum = outpsum_cm.__enter__()

            # ---- per-k-tile prep: z8 = fp8(64*d*Z), rp = fp8(64*d*Z - z8).
            # ACT makes z8, DVE makes the residual; k-ascending so the main
            # matmul (k-pair outer) can stream right behind.
            z8p = [
                persist.tile([128, 2 * F], fp8, name=f"z8_{kp}") for kp in range(KP)
            ]
            rp8 = [
                persist.tile([128, 2 * F], fp8, name=f"rp_{kp}") for kp in range(KP)
            ]
            for m in range(KT):
                kp, half = m // 2, m % 2
                zsl = z_all[:, m * F : (m + 1) * F]
                z8sl = z8p[kp][:, half * F : (half + 1) * F]
                rpsl = rp8[kp][:, half * F : (half + 1) * F]
                nc.scalar.activation(
                    z8sl,
                    zsl,
                    mybir.ActivationFunctionType.Copy,
                    scale=d64[:, m : m + 1],
                )
                nc.vector.scalar_tensor_tensor(
                    out=rpsl,
                    in0=zsl,
                    scalar=d64[:, m : m + 1],
                    in1=z8sl,
                    op0=mybir.AluOpType.mult,
                    op1=mybir.AluOpType.subtract,
                )
                if debug:
                    zs = work.tile([128, F], f32, tag="zdump")
                    nc.vector.tensor_copy(zs[:], z8sl)
                    nc.sync.dma_start(
                        out=z_dump[m * 128 : (m + 1) * 128, :], in_=zs[:]
                    )

            # ---- main matmul, DoubleRow fp8: psum[jt] accumulates
            # A_pair^T @ z8_pair + A_pair^T @ rp_pair over all pairs.
            # k-pair outer / jt inner: all 8 banks host one group each and
            # the PE streams behind the prep above.
            out_ps = [
                outpsum.tile([128, F], f32, name=f"out_ps{jt}", tag=f"out_ps{jt}")
                for jt in range(JT)
            ]
            for kp in range(KP):
                for jt in range(JT):
                    ap = a_pair(kp, jt * 128, (jt + 1) * 128)
                    nc.tensor.matmul(
                        out_ps[jt][:],
                        ap,
                        z8p[kp].rearrange("p (t f) -> p t f", t=2),
                        start=(kp == 0),
                        stop=False,
                        perf_mode=mybir.MatmulPerfMode.DoubleRow,
                    )
                    nc.tensor.matmul(
                        out_ps[jt][:],
                        ap,
                        rp8[kp].rearrange("p (t f) -> p t f", t=2),
                        start=False,
                        stop=(kp == KP - 1),
                        perf_mode=mybir.MatmulPerfMode.DoubleRow,
                    )

            # ---- epilogue: out = (d_own/64) * psum + bias, one STT per jt
            for jt in range(JT):
                ot = work.tile([128, F], f32, tag="ot", bufs=3)
                nc.vector.scalar_tensor_tensor(
                    out=ot[:],
                    in0=out_ps[jt][:],
                    scalar=d_own[:, jt : jt + 1],
                    in1=bias_sb[:],
                    op0=mybir.AluOpType.mult,
                    op1=mybir.AluOpType.add,
                )
                nc.sync.dma_start(out=out[jt * 128 : (jt + 1) * 128, :], in_=ot[:])

            outpsum_cm.__exit__(None, None, None)

    if split_waits:
        _split_dma_waits(nc)
    return nc


_CACHE = {}


def _get_program(n_nodes: int, debug: bool = False):
    key = (n_nodes, debug)
    if key not in _CACHE:
        _CACHE[key] = build(n_nodes, debug=debug)
    return _CACHE[key]


def _prep_inputs(A, inputs, weight, bias):
    """Host-side marshaling: shard + layout + dtype casts."""
    N = A.shape[0]
    C = N // NCORES
    A_aug = np.asarray(A, dtype=np.float32)
    idx = np.arange(N)
    A_aug = A_aug.astype(ml_dtypes.float8_e4m3)
    A_aug[idx, idx] = np.float32(1.0)  # reference adds I; A diag is 0
    xt = np.ascontiguousarray(np.asarray(inputs, dtype=np.float32).T).astype(np.float16)
    w16 = np.asarray(weight, dtype=np.float32).astype(np.float16)
    bias_bc = np.ascontiguousarray(
        np.broadcast_to(np.asarray(bias, dtype=np.float32), (128, F))
    )
    in_maps = [
        {
            "a_strip": np.ascontiguousarray(A_aug[:, c * C : (c + 1) * C]),
            "xt": xt,
            "w": w16,
            "bias_bc": bias_bc,
        }
        for c in range(NCORES)
    ]
    return in_maps


def kernel(A, inputs, weight, bias):
    N = A.shape[0]
    nc = _get_program(N)
    in_maps = _prep_inputs(A, inputs, weight, bias)
    res = run_bass_kernel_spmd(nc, in_maps, list(range(NCORES)))
    return np.concatenate([r["out"] for r in res.results], axis=0)


if __name__ == "__main__":
    # mini self-check with a host reference
    N = 1024
    rng = np.random.default_rng(0)
    A = (rng.random((N, N)) < 0.01).astype(np.float32)
    A = np.maximum(A, A.T)
    np.fill_diagonal(A, 0.0)
    X = rng.standard_normal((N, F)).astype(np.float32)
    W = (rng.random((F, F)).astype(np.float32) / 100.0) - 0.005
    b = (rng.random(F).astype(np.float32) / 100.0) - 0.005

    A_ = A + np.eye(N, dtype=np.float32)
    deg = A_.sum(axis=1)
    d = deg**-0.5
    expected = (d[:, None] * A_ * d[None, :]) @ X @ W + b

    nc = _get_program(N, debug=True)
    in_maps = _prep_inputs(A, X, W, b)
    res = run_bass_kernel_spmd(nc, in_maps, list(range(NCORES)))
    r0 = res.results[0]
    deg_got = r0["deg_dump"]  # [128, KT] col k = deg[k*128:(k+1)*128]
    deg_exp = deg.reshape(-1, 128).T
    print("deg ok:", np.allclose(deg_got, deg_exp))

    got = np.concatenate([r["out"] for r in res.results], axis=0)
    err = np.abs(got - expected)
    scale = np.abs(expected).max()
    print("rel err:", err.max() / scale, "nan:", np.isnan(got).sum(), "/", got.size)
